# revision 1
# baseline (speedup 1.0000x reference)
"""Trainium2 Bass kernel for nn_DiUT_Llama_46901042872838 (moe_routing).

MoE attention: dense sigmoid-gated mixture of E=4 attention experts over
[B=1, S=1024, D=1024], H=16 heads, per-expert QK-layernorm + rope.

Sharding (8 cores): core c -> (expert e = c//2, seq-half j = c%2).
Each core computes, for its expert: full K/V (all S positions), Q for its
512 rows, attention, wo projection and the expert gate -> gated partial
output [512, 1024]. Host sums the 4 expert partials per row block.

Kernel layout notes:
- Activations kept transposed [feature-part, seq-free]. Host passes xT with
  the core's own 512 rows permuted to the front so the SPMD program is
  identical on all cores (K/V consume all rows; softmax is invariant to the
  key-order permutation as long as K and V use the same order).
- LN mean folded into host-centered wq/wk. Variance via squares + ones-matmul
  partition reduction; rstd (with the 1/sqrt(HD) logit scale folded for Q)
  broadcast across partitions via a PE outer product.
- Rope pairs (2i, 2i+1) live on adjacent partitions; the "swapped" operand
  comes from a pair-swap permutation matmul; cos/sin (+/- sign and rstd
  folded) are host-built [128, S] patterns.
- Softmax without max-subtraction (|logit| <= 8 after QK-LN). exp'd logits
  are matmul'd against V extended with a ones column, so each head's
  softmax denominator falls out of the same accumulation (psum row 64);
  the division is applied in the per-head epilogue.
- Big matmuls run as float32r (bitcast), exact fp32 for stats/broadcasts.
"""

import sys

if "/opt/trn_rl_repo" not in sys.path:
    sys.path.insert(0, "/opt/trn_rl_repo")

import numpy as np

E, B, S, D, H = 4, 1, 1024, 1024, 16
HD = D // H          # 64
SQ = S // 2          # query rows per core
N_CORES = 8
DT = 8               # d-dim 128-chunks
EPS = 1e-5

TRACE = False        # test harness sets True to get NTFF timing
LAST_RESULT = None   # BassKernelResults of the most recent run

_compiled = {}


def _build_program():
    import concourse.bacc as bacc
    import concourse.mybir as mybir
    import concourse.tile as tile
    import concourse.bass as bass

    f32 = mybir.dt.float32
    bf16 = mybir.dt.float16
    AF = mybir.ActivationFunctionType

    nc = bacc.Bacc("TRN2", target_bir_lowering=False, debug=False,
                   num_devices=N_CORES)

    # ---- I/O (matmul operands in bf16) ----
    xt_d = nc.dram_tensor("xt", [D, S], bf16, kind="ExternalInput")
    wq_d = nc.dram_tensor("wq", [D, D], bf16, kind="ExternalInput")
    wk_d = nc.dram_tensor("wk", [D, D], bf16, kind="ExternalInput")
    wv_d = nc.dram_tensor("wv", [D, D], bf16, kind="ExternalInput")
    wo_d = nc.dram_tensor("wo", [D, D], bf16, kind="ExternalInput")
    cm_d = nc.dram_tensor("cm", [128, S], bf16, kind="ExternalInput")
    sm_d = nc.dram_tensor("sm", [128, S], bf16, kind="ExternalInput")
    pswap_d = nc.dram_tensor("pswap", [128, 128], bf16, kind="ExternalInput")
    gcol_d = nc.dram_tensor("gcol", [D, 2], bf16, kind="ExternalInput")
    gbias_d = nc.dram_tensor("gbias", [1, 1], f32, kind="ExternalInput")
    selp_d = nc.dram_tensor("selp", [40, 8 * 128], bf16, kind="ExternalInput")
    out_d = nc.dram_tensor("out", [SQ, D], f32, kind="ExternalOutput")

    from contextlib import ExitStack
    with tile.TileContext(nc) as tc, ExitStack() as _es:
        p_x = _es.enter_context(tc.tile_pool(name="p_x", bufs=8))
        p_w = _es.enter_context(tc.tile_pool(name="p_w", bufs=13))
        p_qr = _es.enter_context(tc.tile_pool(name="p_qr", bufs=8))
        p_qn = _es.enter_context(tc.tile_pool(name="p_qn", bufs=8))
        p_kr = _es.enter_context(tc.tile_pool(name="p_kr", bufs=8))
        p_kn = _es.enter_context(tc.tile_pool(name="p_kn", bufs=8))
        p_v = _es.enter_context(tc.tile_pool(name="p_v", bufs=8))
        p_sc = _es.enter_context(tc.tile_pool(name="p_sc", bufs=4))
        p_sq = _es.enter_context(tc.tile_pool(name="p_sq", bufs=3))
        p_e = _es.enter_context(tc.tile_pool(name="p_e", bufs=6))
        p_o = _es.enter_context(tc.tile_pool(name="p_o", bufs=16))
        p_f = _es.enter_context(tc.tile_pool(name="p_f", bufs=2))
        p_g = _es.enter_context(tc.tile_pool(name="p_g", bufs=4))
        p_r = _es.enter_context(tc.tile_pool(name="p_r", bufs=4))
        p_1 = _es.enter_context(tc.tile_pool(name="p_1", bufs=1))
        ps_mm = _es.enter_context(tc.tile_pool(name="ps_mm", bufs=2, space="PSUM"))
        ps_bc = _es.enter_context(tc.tile_pool(name="ps_bc", bufs=2, space="PSUM"))
        ps_acc = _es.enter_context(tc.tile_pool(name="ps_acc", bufs=2, space="PSUM"))
        if True:

            # ---- constants / small inputs ----
            ones_col = p_1.tile([128, 1], bf16, tag="ones_col")
            nc.vector.memset(ones_col[:], 1.0)
            zero_b = p_1.tile([128, 1], f32, tag="zero_b")
            nc.vector.memset(zero_b[:], 0.0)
            eps_q = p_1.tile([1, 1], f32, tag="eps_q")
            nc.vector.memset(eps_q[:], float(HD * EPS))
            eps_k = p_1.tile([1, 1], f32, tag="eps_k")
            nc.vector.memset(eps_k[:], float(EPS))
            pswap_sb = p_1.tile([128, 128], bf16, tag="pswap")
            nc.sync.dma_start(pswap_sb[:], pswap_d[:])
            # PE warm-up burst: ~5us of back-to-back matmuls as soon as the
            # first tiny input lands, so the HAM clock-gate opens before the
            # real weight stream arrives.
            warm = ps_bc.tile([128, 128], f32, tag="bc", name="warm")
            for wi in range(256):
                nc.tensor.matmul(warm[:], pswap_sb[:], pswap_sb[:],
                                 start=True, stop=True)
            cm_sb = p_1.tile([128, S], bf16, tag="cm")
            nc.sync.dma_start(cm_sb[:], cm_d[:])
            sm_sb = p_1.tile([128, S], bf16, tag="sm")
            nc.sync.dma_start(sm_sb[:], sm_d[:])
            gcol_sb = p_1.tile([128, 16], bf16, tag="gcol")
            nc.sync.dma_start(
                gcol_sb[:].rearrange("p (k o) -> p k o", o=2),
                gcol_d[:].rearrange("(k p) o -> p k o", p=128))
            gb_sb = p_1.tile([128, 1], f32, tag="gb")
            gb_bcast = bass.AP(tensor=gbias_d, offset=0, ap=[[0, 128], [1, 1]])
            nc.sync.dma_start(gb_sb[:], gb_bcast)
            # selector for rstd broadcasts: sel3[:, i*128:(i+1)*128] picks row i
            sel3 = p_1.tile([65, 3 * 128], bf16, tag="sel3")
            nc.vector.memset(sel3[:], 0.0)
            for i in range(3):
                nc.vector.memset(
                    sel3[32 * i:32 * i + 1, i * 128:(i + 1) * 128], 1.0)
            # selector for per-headpair recip broadcast (host-built):
            # block p: col m -> row 2p for m<64 else row 2p+1
            selp = p_1.tile([40, 8 * 128], bf16, tag="selp")
            nc.sync.dma_start(selp[:], selp_d[:])

            # ---- x (transposed, core-permuted) ----
            xt_sb = []
            for k in range(DT):
                t = p_x.tile([128, S], bf16, tag="x", name=f"xt{k}")
                nc.sync.dma_start(t[:], xt_d[k * 128:(k + 1) * 128, :])
                xt_sb.append(t)

            # ---- stats psum: row 0 = q sumsq, rows 32/64 = k halves ----
            stats = ps_bc.tile([65, 512], f32, tag="bc")

            # ================= Phase A: projections =================
            # --- Q projection (own 512 rows) + stats; 2 m-tiles per psum ---
            wq_sb = []
            for k in range(DT):
                t = p_w.tile([128, D], bf16, tag="w", name=f"wq{k}")
                nc.sync.dma_start(t[:], wq_d[k * 128:(k + 1) * 128, :])
                wq_sb.append(t)

            q_raw = []
            for mp in range(4):
                pq = ps_mm.tile([128, 1024], f32, tag="mm", name=f"pq{mp}")
                for half in range(2):
                    m = 2 * mp + half
                    hs = slice(half * 512, (half + 1) * 512)
                    for k in range(DT):
                        nc.tensor.matmul(
                            pq[:, hs], wq_sb[k][:, m * 128:(m + 1) * 128],
                            xt_sb[k][:, 0:SQ],
                            start=(k == 0), stop=(k == DT - 1))
                    qr = p_qr.tile([128, 512], bf16, tag="qr",
                                   name=f"qraw{m}")
                    nc.vector.tensor_copy(qr[:], pq[:, hs])
                    sq = p_sq.tile([128, 512], bf16, tag="sq",
                                   name=f"sqq{m}")
                    nc.scalar.activation(sq[:], pq[:, hs], AF.Square,
                                         bias=zero_b[:])
                    nc.tensor.matmul(stats[0:1, :], ones_col[:], sq[:],
                                     start=(m == 0), stop=(m == 7))
                    q_raw.append(qr)

            # --- K projection (all rows) + stats ---
            wk_sb = []
            for k in range(DT):
                t = p_w.tile([128, D], bf16, tag="w", name=f"wk{k}")
                nc.sync.dma_start(t[:], wk_d[k * 128:(k + 1) * 128, :])
                wk_sb.append(t)

            k_raw = []
            for m in range(8):
                pk = ps_mm.tile([128, 1024], f32, tag="mm", name=f"pk{m}")
                for nb in range(2):
                    hs = slice(nb * 512, (nb + 1) * 512)
                    for k in range(DT):
                        nc.tensor.matmul(
                            pk[:, hs], wk_sb[k][:, m * 128:(m + 1) * 128],
                            xt_sb[k][:, hs],
                            start=(k == 0), stop=(k == DT - 1))
                kr = p_kr.tile([128, S], bf16, tag="kr", name=f"kraw{m}")
                nc.vector.tensor_copy(kr[:], pk[:])
                for nb in range(2):
                    hs = slice(nb * 512, (nb + 1) * 512)
                    sq = p_sq.tile([128, 512], bf16, tag="sq",
                                   name=f"sqk{m}_{nb}")
                    nc.scalar.activation(sq[:], pk[:, hs], AF.Square,
                                         bias=zero_b[:])
                    r0 = 32 + 32 * nb
                    nc.tensor.matmul(stats[r0:r0 + 1, :], ones_col[:], sq[:],
                                     start=(m == 0), stop=(m == 7))
                k_raw.append(kr)

            # --- rstd (batched): rows 0/32/64 = q, k0, k1 ---
            r3s = p_r.tile([65, 512], f32, tag="r", name="r3s")
            nc.vector.memset(r3s[:], 1.0)
            nc.scalar.activation(r3s[0:1, :], stats[0:1, :], AF.Sqrt,
                                 bias=eps_q[:], scale=float(HD) / D)
            nc.scalar.activation(r3s[32:33, :], stats[32:33, :], AF.Sqrt,
                                 bias=eps_k[:], scale=1.0 / D)
            nc.scalar.activation(r3s[64:65, :], stats[64:65, :], AF.Sqrt,
                                 bias=eps_k[:], scale=1.0 / D)
            r3 = p_r.tile([65, 512], bf16, tag="r3", name="r3")
            with nc.allow_low_precision(reason="rstd fits fp16"):
                nc.vector.reciprocal(r3[:], r3s[:])

            # --- rope multipliers with rstd folded (broadcast via PE) ---
            cmq = p_1.tile([128, 512], bf16, tag="cmq")
            smq = p_1.tile([128, 512], bf16, tag="smq")
            cmk = p_1.tile([128, S], bf16, tag="cmk")
            smk = p_1.tile([128, S], bf16, tag="smk")
            bcq = ps_bc.tile([128, 512], f32, tag="bc", name="bcq")
            nc.tensor.matmul(bcq[:], sel3[:, 0:128], r3[:],
                             start=True, stop=True)
            nc.vector.tensor_mul(cmq[:], cm_sb[:, 0:SQ], bcq[:])
            nc.vector.tensor_mul(smq[:], sm_sb[:, 0:SQ], bcq[:])
            for nb in range(2):
                sl = slice(nb * 512, (nb + 1) * 512)
                bck = ps_bc.tile([128, 512], f32, tag="bc", name=f"bck{nb}")
                nc.tensor.matmul(bck[:], sel3[:, (1 + nb) * 128:(2 + nb) * 128],
                                 r3[:], start=True, stop=True)
                nc.vector.tensor_mul(cmk[:, sl], cm_sb[:, sl], bck[:])
                nc.vector.tensor_mul(smk[:, sl], sm_sb[:, sl], bck[:])

            # ================= Phase B: attention =================
            # --- V projection into v_ext (bf16, ones col per head) ---
            wv_sb = []
            for k in range(DT):
                t = p_w.tile([128, D], bf16, tag="w", name=f"wv{k}")
                nc.sync.dma_start(t[:], wv_d[k * 128:(k + 1) * 128, :])
                wv_sb.append(t)

            v_ext = []
            for tch in range(8):
                vx = p_v.tile([128, H * (HD + 1)], bf16, tag="v",
                              name=f"vext{tch}")
                vx3 = vx[:].rearrange("p (h c) -> p h c", c=HD + 1)
                nc.vector.memset(vx3[:, :, HD:HD + 1], 1.0)
                pv = ps_mm.tile([128, 1024], f32, tag="mm", name=f"pv{tch}")
                for nb in range(2):
                    hs = slice(nb * 512, (nb + 1) * 512)
                    for k in range(DT):
                        nc.tensor.matmul(
                            pv[:, hs], xt_sb[k][:, tch * 128:(tch + 1) * 128],
                            wv_sb[k][:, hs],
                            start=(k == 0), stop=(k == DT - 1))
                dst = vx3[:, :, 0:HD]
                src = pv[:].rearrange("p (h c) -> p h c", c=HD)
                nc.vector.tensor_copy(dst, src)
                v_ext.append(vx)

            wo_sb = []
            for k in range(DT):
                t = p_w.tile([128, D], bf16, tag="w", name=f"wo{k}")
                nc.sync.dma_start(t[:], wo_d[k * 128:(k + 1) * 128, :])
                wo_sb.append(t)

            # --- gate (own rows, this expert's column) ---
            gate_sb = []
            for sc in range(4):
                pg = ps_bc.tile([128, 2], f32, tag="bc", name=f"pg{sc}")
                for k in range(DT):
                    nc.tensor.matmul(pg[:],
                                     xt_sb[k][:, sc * 128:(sc + 1) * 128],
                                     gcol_sb[:, 2 * k:2 * k + 2],
                                     start=(k == 0), stop=(k == DT - 1))
                g = p_1.tile([128, 1], f32, tag=f"gate{sc}", name=f"gate{sc}")
                nc.scalar.activation(g[:], pg[:, 0:1], AF.Sigmoid,
                                     bias=gb_sb[:, 0:1], scale=1.0)
                gate_sb.append(g)

            # --- fused rope + attention, per head pair ---
            outU = [p_o.tile([128, 512], bf16, tag="o", name=f"outU{i}")
                    for i in range(8)]
            outT = [p_o.tile([128, 512], bf16, tag="o", name=f"outT{i}")
                    for i in range(8)]
            p1g = [p_g.tile([128, 1024], bf16, tag="p1g", name=f"p1g{i}")
                   for i in range(4)]
            sume = p_1.tile([40, 512], f32, tag="sume")
            rall = p_1.tile([40, 512], bf16, tag="rall")
            nc.vector.memset(rall[:], 1.0)
            xqn = []
            xkn = []
            for m in range(8):
                psw = ps_bc.tile([128, 512], f32, tag="bc", name=f"pswq{m}")
                nc.tensor.matmul(psw[:], pswap_sb[:], q_raw[m][:],
                                 start=True, stop=True)
                t2 = p_sc.tile([128, 512], bf16, tag="sc", name=f"qt2_{m}")
                nc.vector.tensor_mul(t2[:], psw[:], smq[:])
                t1 = p_sc.tile([128, 512], bf16, tag="sc", name=f"qt1_{m}")
                nc.vector.tensor_mul(t1[:], q_raw[m][:], cmq[:])
                qn = p_qn.tile([128, 512], bf16, tag="qn", name=f"xqn{m}")
                nc.vector.tensor_add(qn[:], t1[:], t2[:])
                xqn.append(qn)
                kn = p_kn.tile([128, S], bf16, tag="kn", name=f"xkn{m}")
                for nb in range(2):
                    sl = slice(nb * 512, (nb + 1) * 512)
                    psw = ps_bc.tile([128, 512], f32, tag="bc",
                                     name=f"pswk{m}_{nb}")
                    nc.tensor.matmul(psw[:], pswap_sb[:], k_raw[m][:, sl],
                                     start=True, stop=True)
                    t2 = p_sc.tile([128, 512], bf16, tag="sc",
                                   name=f"kt2_{m}_{nb}")
                    nc.vector.tensor_mul(t2[:], psw[:], smk[:, sl])
                    t1 = p_sc.tile([128, 512], bf16, tag="sc",
                                   name=f"kt1_{m}_{nb}")
                    nc.vector.tensor_mul(t1[:], k_raw[m][:, sl], cmk[:, sl])
                    nc.vector.tensor_add(kn[:, sl], t1[:], t2[:])
                xkn.append(kn)
                p = m
                oacc = [ps_acc.tile([HD + 1, 512], f32, tag="acc",
                                    name=f"oacc{p}_{i}") for i in range(2)]
                for tch in range(8):
                    pl = ps_mm.tile([128, 1024], f32, tag="mm",
                                    name=f"pl{p}_{tch}")
                    for idx in range(2):
                        base = 64 * idx
                        nc.tensor.matmul(
                            pl[:, idx * 512:(idx + 1) * 512],
                            xkn[p][base:base + 64,
                                   tch * 128:(tch + 1) * 128],
                            xqn[p][base:base + 64, :],
                            start=True, stop=True)
                    ex = p_e.tile([128, 1024], bf16, tag="e",
                                  name=f"ex{p}_{tch}")
                    nc.scalar.activation(ex[:], pl[:], AF.Exp,
                                         bias=zero_b[:])
                    for idx in range(2):
                        h = 2 * p + idx
                        nc.tensor.matmul(
                            oacc[idx][:],
                            v_ext[tch][:, h * (HD + 1):(h + 1) * (HD + 1)],
                            ex[:, idx * 512:(idx + 1) * 512],
                            start=(tch == 0), stop=(tch == 7))
                for idx in range(2):
                    base = 64 * idx
                    h = 2 * p + idx
                    nc.vector.tensor_copy(outU[p][base:base + 64, :],
                                          oacc[idx][0:HD, :])
                    se = p_r.tile([1, 512], f32, tag="se",
                                  name=f"se{p}_{idx}", bufs=4)
                    nc.vector.tensor_copy(se[:], oacc[idx][HD:HD + 1, :])
                    row = h if p < 4 else 32 + (h - 8)
                    nc.sync.dma_start(sume[row:row + 1, :], se[:])
                if p == 3:
                    with nc.allow_low_precision(reason="softmax denom fp16"):
                        nc.vector.reciprocal(rall[0:8, :], sume[0:8, :])
                    for p2 in range(4):
                        bc = ps_bc.tile([128, 512], f32, tag="bc",
                                        name=f"bcr{p2}")
                        nc.tensor.matmul(bc[:],
                                         selp[0:8, p2 * 128:(p2 + 1) * 128],
                                         rall[0:8, :], start=True, stop=True)
                        nc.vector.tensor_mul(outT[p2][:], outU[p2][:], bc[:])
                if p == 4:
                    # wo half-contraction over first 4 outT tiles
                    for sc in range(4):
                        pf = ps_mm.tile([128, 1024], f32, tag="mm",
                                        name=f"pf1_{sc}")
                        for fb in range(2):
                            hs = slice(fb * 512, (fb + 1) * 512)
                            for cc in range(4):
                                nc.tensor.matmul(
                                    pf[:, hs],
                                    outT[cc][:, sc * 128:(sc + 1) * 128],
                                    wo_sb[cc][:, hs],
                                    start=(cc == 0), stop=(cc == 3))
                        nc.vector.tensor_scalar_mul(p1g[sc][:], pf[:],
                                                    gate_sb[sc][:])
                if p == 7:
                    with nc.allow_low_precision(reason="softmax denom fp16"):
                        nc.vector.reciprocal(rall[32:40, :], sume[32:40, :])

            # softmax denominators for pairs 4-7 (0-3 handled mid-loop)
            for p in range(4, 8):
                bc = ps_bc.tile([128, 512], f32, tag="bc", name=f"bcr{p}")
                nc.tensor.matmul(bc[:],
                                 selp[32:40, p * 128:(p + 1) * 128],
                                 rall[32:40, :], start=True, stop=True)
                nc.vector.tensor_mul(outT[p][:], outU[p][:], bc[:])

            # ================= Phase C: wo projection + gate =================
            for sc in range(4):
                pf = ps_mm.tile([128, 1024], f32, tag="mm", name=f"pf2_{sc}")
                for fb in range(2):
                    hs = slice(fb * 512, (fb + 1) * 512)
                    for cc in range(4, 8):
                        nc.tensor.matmul(
                            pf[:, hs], outT[cc][:, sc * 128:(sc + 1) * 128],
                            wo_sb[cc][:, hs],
                            start=(cc == 4), stop=(cc == 7))
                fin = p_f.tile([128, 1024], f32, tag="f", name=f"fin{sc}")
                nc.vector.scalar_tensor_tensor(
                    fin[:], pf[:], gate_sb[sc][:], p1g[sc][:],
                    op0=mybir.AluOpType.mult, op1=mybir.AluOpType.add)
                nc.sync.dma_start(out_d[sc * 128:(sc + 1) * 128, :], fin[:])

    nc.compile()
    return nc


def _get_program():
    if "nc" not in _compiled:
        _compiled["nc"] = _build_program()
    return _compiled["nc"]


def _host_prep(inputs):
    """Build the 8 per-core input maps."""
    x = np.asarray(inputs["x"], np.float32).reshape(S, D)
    fc = np.asarray(inputs["freqs_cos"], np.float32)   # [S, HD//2]
    fs = np.asarray(inputs["freqs_sin"], np.float32)
    wq = np.asarray(inputs["wq"], np.float32)
    wk = np.asarray(inputs["wk"], np.float32)
    wv = np.asarray(inputs["wv"], np.float32)
    wo = np.asarray(inputs["wo"], np.float32)
    gate_w = np.asarray(inputs["gate_w"], np.float32)
    gate_b = np.asarray(inputs["gate_b"], np.float32)

    # centered LN weights (exact mean-subtraction fold)
    wq_c = wq - wq.mean(axis=2, keepdims=True)
    wk_c = wk - wk.mean(axis=2, keepdims=True)

    # rope partition patterns: p -> freq index (p%64)//2, sign -1 even/+1 odd
    p_idx = np.arange(128)
    fidx = (p_idx % 64) // 2
    sign = np.where(p_idx % 2 == 0, -1.0, 1.0).astype(np.float32)
    # [128, S] patterns in original position order
    cm_full = fc[:, fidx].T.copy()                    # [128, S]
    sm_full = (fs[:, fidx].T * sign[:, None]).copy()  # [128, S]

    pswap = np.zeros((128, 128), np.float32)
    pswap[p_idx, p_idx ^ 1] = 1.0
    pswap = pswap.astype(np.float16)

    selp = np.zeros((40, 8 * 128), np.float32)
    for p in range(8):
        r0, r1 = (2 * p, 2 * p + 1) if p < 4 else (24 + 2 * p, 25 + 2 * p)
        selp[r0, p * 128:p * 128 + 64] = 1.0
        selp[r1, p * 128 + 64:(p + 1) * 128] = 1.0

    in_maps = []
    for c in range(N_CORES):
        e, j = c // 2, c % 2
        perm = np.concatenate([np.arange(j * SQ, (j + 1) * SQ),
                               np.arange((1 - j) * SQ, (2 - j) * SQ)])
        xt = np.ascontiguousarray(x[perm].T)          # [D, S]
        bf = np.float16
        in_maps.append({
            "xt": xt.astype(bf),
            "wq": np.ascontiguousarray(wq_c[e]).astype(bf),
            "wk": np.ascontiguousarray(wk_c[e]).astype(bf),
            "wv": np.ascontiguousarray(wv[e]).astype(bf),
            "wo": np.ascontiguousarray(wo[e]).astype(bf),
            "cm": np.ascontiguousarray(cm_full[:, perm]).astype(bf),
            "sm": np.ascontiguousarray(sm_full[:, perm]).astype(bf),
            "pswap": pswap,
            "gcol": np.ascontiguousarray(
                np.concatenate([gate_w[:, e:e + 1],
                                np.zeros((D, 1), np.float32)],
                               axis=1)).astype(bf),
            "gbias": gate_b[e].reshape(1, 1),
            "selp": selp.astype(np.float16),
        })
    return in_maps


def _trivial_ln_params(inputs):
    return (np.allclose(np.asarray(inputs["q_gamma"]), 1.0)
            and np.allclose(np.asarray(inputs["k_gamma"]), 1.0)
            and np.allclose(np.asarray(inputs["q_beta"]), 0.0)
            and np.allclose(np.asarray(inputs["k_beta"]), 0.0))


def _numpy_fallback(inputs):
    """Exact reference math on host; only used for nontrivial gamma/beta
    (never hit for this problem's input spec: gamma==1, beta==0)."""
    x = np.asarray(inputs["x"], np.float64)
    fc = np.asarray(inputs["freqs_cos"], np.float64)
    fs = np.asarray(inputs["freqs_sin"], np.float64)
    wq = np.asarray(inputs["wq"], np.float64)
    wk = np.asarray(inputs["wk"], np.float64)
    wv = np.asarray(inputs["wv"], np.float64)
    wo = np.asarray(inputs["wo"], np.float64)
    qg = np.asarray(inputs["q_gamma"], np.float64)
    qb = np.asarray(inputs["q_beta"], np.float64)
    kg = np.asarray(inputs["k_gamma"], np.float64)
    kb = np.asarray(inputs["k_beta"], np.float64)
    gw = np.asarray(inputs["gate_w"], np.float64)
    gb = np.asarray(inputs["gate_b"], np.float64)

    def ln(v, g, b):
        m = v.mean(-1, keepdims=True)
        va = ((v - m) ** 2).mean(-1, keepdims=True)
        return (v - m) / np.sqrt(va + EPS) * g + b

    def rope(q):
        qr = q.reshape(q.shape[:-1] + (HD // 2, 2))
        a, b = qr[..., 0], qr[..., 1]
        c = fc[None, None, :, None, :]
        s = fs[None, None, :, None, :]
        return np.stack([a * c - b * s, a * s + b * c], -1).reshape(q.shape)

    gate = 1.0 / (1.0 + np.exp(-(x @ gw + gb)))
    xq = np.einsum("bsd,edh->ebsh", x, wq)
    xk = np.einsum("bsd,edh->ebsh", x, wk)
    xv = np.einsum("bsd,edh->ebsh", x, wv)
    xq = ln(xq, qg[:, None, None, :], qb[:, None, None, :])
    xk = ln(xk, kg[:, None, None, :], kb[:, None, None, :])
    xq = rope(xq.reshape(E, B, S, H, HD))
    xk = rope(xk.reshape(E, B, S, H, HD))
    xv = xv.reshape(E, B, S, H, HD)
    lg = np.einsum("ebshk,ebthk->ebhst", xq, xk) / np.sqrt(HD)
    lg = np.exp(lg - lg.max(-1, keepdims=True))
    at = lg / lg.sum(-1, keepdims=True)
    o = np.einsum("ebhst,ebthk->ebshk", at, xv).reshape(E, B, S, D)
    o = np.einsum("ebsd,edf->ebsf", o, wo)
    return np.einsum("ebsd,bse->bsd", o, gate).astype(np.float32)


def kernel(**inputs):
    global LAST_RESULT
    if not _trivial_ln_params(inputs):
        return _numpy_fallback(inputs)

    from concourse import bass_utils

    nc = _get_program()
    in_maps = _host_prep(inputs)
    res = bass_utils.run_bass_kernel_spmd(
        nc, in_maps, core_ids=list(range(N_CORES)), trace=TRACE)
    LAST_RESULT = res

    out = np.zeros((S, D), np.float32)
    for c in range(N_CORES):
        j = c % 2
        out[j * SQ:(j + 1) * SQ] += res.results[c]["out"]
    return out.reshape(B, S, D)



# revision 8
# speedup vs baseline: 1.1831x; 1.1831x over previous
"""Trainium2 Bass kernel for nn_DiUT_Llama_46901042872838 (moe_routing).

MoE attention: dense sigmoid-gated mixture of E=4 attention experts over
[B=1, S=1024, D=1024], H=16 heads, per-expert QK-layernorm + rope.

Sharding (8 cores): core c -> (expert e = c//2, seq-half j = c%2).
Each core computes, for its expert: full K/V (all S positions), Q for its
512 rows, attention, wo projection and the expert gate -> gated partial
output [512, 1024]. Host sums the 4 expert partials per row block.

Kernel layout notes:
- Activations kept transposed [feature-part, seq-free]. Host passes xT with
  the core's own 512 rows permuted to the front so the SPMD program is
  identical on all cores (K/V consume all rows; softmax is invariant to the
  key-order permutation as long as K and V use the same order).
- LN mean folded into host-centered wq/wk. Variance via squares + ones-matmul
  partition reduction; rstd (with the 1/sqrt(HD) logit scale folded for Q)
  broadcast across partitions via a PE outer product.
- Rope pairs (2i, 2i+1) live on adjacent partitions; the "swapped" operand
  comes from a pair-swap permutation matmul; cos/sin (+/- sign and rstd
  folded) are host-built [128, S] patterns.
- Softmax without max-subtraction (|logit| <= 8 after QK-LN). exp'd logits
  are matmul'd against V extended with a ones column, so each head's
  softmax denominator falls out of the same accumulation (psum row 64);
  the division is applied in the per-head epilogue.
- Big matmuls run as float32r (bitcast), exact fp32 for stats/broadcasts.
"""

import sys

if "/opt/trn_rl_repo" not in sys.path:
    sys.path.insert(0, "/opt/trn_rl_repo")

import numpy as np

E, B, S, D, H = 4, 1, 1024, 1024, 16
HD = D // H          # 64
SQ = S // 2          # query rows per core
N_CORES = 8
DT = 8               # d-dim 128-chunks
EPS = 1e-5

TRACE = False        # test harness sets True to get NTFF timing
LAST_RESULT = None   # BassKernelResults of the most recent run

_compiled = {}


def _build_program():
    import concourse.bacc as bacc
    import concourse.mybir as mybir
    import concourse.tile as tile
    import concourse.bass as bass

    f32 = mybir.dt.float32
    bf16 = mybir.dt.float16
    AF = mybir.ActivationFunctionType

    nc = bacc.Bacc("TRN2", target_bir_lowering=False, debug=False,
                   num_devices=N_CORES)

    # ---- I/O (matmul operands in bf16) ----
    xt_d = nc.dram_tensor("xt", [D, S], bf16, kind="ExternalInput")
    wq_d = nc.dram_tensor("wq", [D, D], bf16, kind="ExternalInput")
    wk_d = nc.dram_tensor("wk", [D, D], bf16, kind="ExternalInput")
    wv_d = nc.dram_tensor("wv", [D, D], bf16, kind="ExternalInput")
    wo_d = nc.dram_tensor("wo", [D, D], bf16, kind="ExternalInput")
    cm_d = nc.dram_tensor("cm", [128, S], bf16, kind="ExternalInput")
    sm_d = nc.dram_tensor("sm", [128, S], bf16, kind="ExternalInput")
    pswap_d = nc.dram_tensor("pswap", [128, 128], bf16, kind="ExternalInput")
    gcol_d = nc.dram_tensor("gcol", [D, 2], bf16, kind="ExternalInput")
    gbias_d = nc.dram_tensor("gbias", [1, 1], f32, kind="ExternalInput")
    selp_d = nc.dram_tensor("selp", [40, 8 * 128], bf16, kind="ExternalInput")
    out_d = nc.dram_tensor("out", [SQ, D], f32, kind="ExternalOutput")

    from contextlib import ExitStack
    with tile.TileContext(nc) as tc, ExitStack() as _es:
        p_x = _es.enter_context(tc.tile_pool(name="p_x", bufs=8))
        p_w = _es.enter_context(tc.tile_pool(name="p_w", bufs=13))
        p_qr = _es.enter_context(tc.tile_pool(name="p_qr", bufs=8))
        p_qn = _es.enter_context(tc.tile_pool(name="p_qn", bufs=8))
        p_kr = _es.enter_context(tc.tile_pool(name="p_kr", bufs=8))
        p_kn = _es.enter_context(tc.tile_pool(name="p_kn", bufs=8))
        p_v = _es.enter_context(tc.tile_pool(name="p_v", bufs=8))
        p_sc = _es.enter_context(tc.tile_pool(name="p_sc", bufs=4))
        p_sq = _es.enter_context(tc.tile_pool(name="p_sq", bufs=3))
        p_e = _es.enter_context(tc.tile_pool(name="p_e", bufs=6))
        p_o = _es.enter_context(tc.tile_pool(name="p_o", bufs=16))
        p_f = _es.enter_context(tc.tile_pool(name="p_f", bufs=2))
        p_g = _es.enter_context(tc.tile_pool(name="p_g", bufs=4))
        p_r = _es.enter_context(tc.tile_pool(name="p_r", bufs=4))
        p_1 = _es.enter_context(tc.tile_pool(name="p_1", bufs=1))
        ps_mm = _es.enter_context(tc.tile_pool(name="ps_mm", bufs=2, space="PSUM"))
        ps_bc = _es.enter_context(tc.tile_pool(name="ps_bc", bufs=2, space="PSUM"))
        ps_acc = _es.enter_context(tc.tile_pool(name="ps_acc", bufs=2, space="PSUM"))
        if True:

            # PE warm-up burst on a memset tile (no DMA dependency): keeps
            # the HAM clock-gate open while the weight stream arrives.
            warm_sb = p_1.tile([128, 512], bf16, tag="warm_sb")
            nc.vector.memset(warm_sb[:], 0.0)
            warm = ps_bc.tile([128, 512], f32, tag="bc", name="warm")
            for wi in range(56):
                nc.tensor.matmul(warm[:], warm_sb[:, 0:128], warm_sb[:],
                                 start=True, stop=True)

            # ---- critical-path DMAs first: x then wq ----
            xt_sb = []
            for k in range(DT):
                t = p_x.tile([128, S], bf16, tag="x", name=f"xt{k}")
                nc.sync.dma_start(t[:], xt_d[k * 128:(k + 1) * 128, :])
                xt_sb.append(t)
            wq_sb = []
            for k in range(DT):
                t = p_w.tile([128, D], bf16, tag="w", name=f"wq{k}")
                nc.sync.dma_start(t[:], wq_d[k * 128:(k + 1) * 128, :])
                wq_sb.append(t)

            # ---- constants / small inputs (needed from rstd onwards) ----
            ones_col = p_1.tile([128, 1], bf16, tag="ones_col")
            nc.vector.memset(ones_col[:], 1.0)
            zero_b = p_1.tile([128, 1], f32, tag="zero_b")
            nc.vector.memset(zero_b[:], 0.0)
            eps_q = p_1.tile([1, 1], f32, tag="eps_q")
            nc.vector.memset(eps_q[:], float(HD * EPS))
            eps_k = p_1.tile([1, 1], f32, tag="eps_k")
            nc.vector.memset(eps_k[:], float(EPS))
            pswap_sb = p_1.tile([128, 128], bf16, tag="pswap")
            nc.sync.dma_start(pswap_sb[:], pswap_d[:])
            cm_sb = p_1.tile([128, S], bf16, tag="cm")
            nc.sync.dma_start(cm_sb[:], cm_d[:])
            sm_sb = p_1.tile([128, S], bf16, tag="sm")
            nc.sync.dma_start(sm_sb[:], sm_d[:])
            gcol_sb = p_1.tile([128, 16], bf16, tag="gcol")
            nc.sync.dma_start(
                gcol_sb[:].rearrange("p (k o) -> p k o", o=2),
                gcol_d[:].rearrange("(k p) o -> p k o", p=128))
            gb_sb = p_1.tile([128, 1], f32, tag="gb")
            gb_bcast = bass.AP(tensor=gbias_d, offset=0, ap=[[0, 128], [1, 1]])
            nc.sync.dma_start(gb_sb[:], gb_bcast)
            # selector for rstd broadcasts: sel3[:, i*128:(i+1)*128] picks row i
            sel3 = p_1.tile([65, 3 * 128], bf16, tag="sel3")
            nc.vector.memset(sel3[:], 0.0)
            for i in range(3):
                nc.vector.memset(
                    sel3[32 * i:32 * i + 1, i * 128:(i + 1) * 128], 1.0)
            # selector for per-headpair recip broadcast (host-built):
            # block p: col m -> row 2p for m<64 else row 2p+1
            selp = p_1.tile([40, 8 * 128], bf16, tag="selp")
            nc.sync.dma_start(selp[:], selp_d[:])

            # ---- stats psum: row 0 = q sumsq, rows 32/64 = k halves.
            # memset so unwritten rows stay at 1.0 (ln/exp keep them finite).
            stats = ps_bc.tile([65, 512], f32, tag="bc")
            nc.vector.memset(stats[:], 1.0)

            # ================= Phase A: projections =================
            # --- Q projection (own 512 rows) + stats; 2 m-tiles per psum ---

            q_raw = []
            for mp in range(4):
                pq = ps_mm.tile([128, 1024], f32, tag="mm", name=f"pq{mp}")
                for half in range(2):
                    m = 2 * mp + half
                    hs = slice(half * 512, (half + 1) * 512)
                    for k in range(DT):
                        nc.tensor.matmul(
                            pq[:, hs], wq_sb[k][:, m * 128:(m + 1) * 128],
                            xt_sb[k][:, 0:SQ],
                            start=(k == 0), stop=(k == DT - 1))
                    qr = p_qr.tile([128, 512], bf16, tag="qr",
                                   name=f"qraw{m}")
                    nc.vector.tensor_copy(qr[:], pq[:, hs])
                    sq = p_sq.tile([128, 512], bf16, tag="sq",
                                   name=f"sqq{m}")
                    nc.scalar.activation(sq[:], pq[:, hs], AF.Square,
                                         bias=zero_b[:])
                    nc.tensor.matmul(stats[0:1, :], ones_col[:], sq[:],
                                     start=(m == 0), stop=(m == 7))
                    q_raw.append(qr)

            # --- K projection (all rows) + stats ---
            wk_sb = []
            for k in range(DT):
                t = p_w.tile([128, D], bf16, tag="w", name=f"wk{k}")
                nc.sync.dma_start(t[:], wk_d[k * 128:(k + 1) * 128, :])
                wk_sb.append(t)

            k_raw = []
            for m in range(8):
                pk = ps_mm.tile([128, 1024], f32, tag="mm", name=f"pk{m}")
                for nb in range(2):
                    hs = slice(nb * 512, (nb + 1) * 512)
                    for k in range(DT):
                        nc.tensor.matmul(
                            pk[:, hs], wk_sb[k][:, m * 128:(m + 1) * 128],
                            xt_sb[k][:, hs],
                            start=(k == 0), stop=(k == DT - 1))
                kr = p_kr.tile([128, S], bf16, tag="kr", name=f"kraw{m}")
                nc.vector.tensor_copy(kr[:], pk[:])
                for nb in range(2):
                    hs = slice(nb * 512, (nb + 1) * 512)
                    sq = p_sq.tile([128, 512], bf16, tag="sq",
                                   name=f"sqk{m}_{nb}")
                    nc.scalar.activation(sq[:], pk[:, hs], AF.Square,
                                         bias=zero_b[:])
                    r0 = 32 + 32 * nb
                    nc.tensor.matmul(stats[r0:r0 + 1, :], ones_col[:], sq[:],
                                     start=(m == 0), stop=(m == 7))
                k_raw.append(kr)

            # --- rstd (batched): rows 0/32/64 = q, k0, k1 ---
            r3s = p_r.tile([65, 512], f32, tag="r", name="r3s")
            nc.vector.memset(r3s[:], 1.0)
            nc.scalar.activation(r3s[0:1, :], stats[0:1, :], AF.Sqrt,
                                 bias=eps_q[:], scale=float(HD) / D)
            nc.scalar.activation(r3s[32:33, :], stats[32:33, :], AF.Sqrt,
                                 bias=eps_k[:], scale=1.0 / D)
            nc.scalar.activation(r3s[64:65, :], stats[64:65, :], AF.Sqrt,
                                 bias=eps_k[:], scale=1.0 / D)
            r3 = p_r.tile([65, 512], bf16, tag="r3", name="r3")
            with nc.allow_low_precision(reason="rstd fits fp16"):
                nc.vector.reciprocal(r3[:], r3s[:])

            # --- rope multipliers with rstd folded (broadcast via PE) ---
            cmq = p_1.tile([128, 512], bf16, tag="cmq")
            smq = p_1.tile([128, 512], bf16, tag="smq")
            cmk = p_1.tile([128, S], bf16, tag="cmk")
            smk = p_1.tile([128, S], bf16, tag="smk")
            bcq = ps_bc.tile([128, 512], f32, tag="bc", name="bcq")
            nc.tensor.matmul(bcq[:], sel3[:, 0:128], r3[:],
                             start=True, stop=True)
            nc.vector.tensor_mul(cmq[:], cm_sb[:, 0:SQ], bcq[:])
            nc.vector.tensor_mul(smq[:], sm_sb[:, 0:SQ], bcq[:])
            for nb in range(2):
                sl = slice(nb * 512, (nb + 1) * 512)
                bck = ps_bc.tile([128, 512], f32, tag="bc", name=f"bck{nb}")
                nc.tensor.matmul(bck[:], sel3[:, (1 + nb) * 128:(2 + nb) * 128],
                                 r3[:], start=True, stop=True)
                nc.vector.tensor_mul(cmk[:, sl], cm_sb[:, sl], bck[:])
                nc.vector.tensor_mul(smk[:, sl], sm_sb[:, sl], bck[:])

            # ================= Phase B: attention =================
            # --- V projection into v_ext (bf16, ones col per head) ---
            wv_sb = []
            for k in range(DT):
                t = p_w.tile([128, D], bf16, tag="w", name=f"wv{k}")
                nc.sync.dma_start(t[:], wv_d[k * 128:(k + 1) * 128, :])
                wv_sb.append(t)

            v_ext = []
            for tch in range(8):
                vx = p_v.tile([128, H * (HD + 1)], bf16, tag="v",
                              name=f"vext{tch}")
                vx3 = vx[:].rearrange("p (h c) -> p h c", c=HD + 1)
                nc.vector.memset(vx3[:, :, HD:HD + 1], 1.0)
                pv = ps_mm.tile([128, 1024], f32, tag="mm", name=f"pv{tch}")
                for nb in range(2):
                    hs = slice(nb * 512, (nb + 1) * 512)
                    for k in range(DT):
                        nc.tensor.matmul(
                            pv[:, hs], xt_sb[k][:, tch * 128:(tch + 1) * 128],
                            wv_sb[k][:, hs],
                            start=(k == 0), stop=(k == DT - 1))
                dst = vx3[:, :, 0:HD]
                src = pv[:].rearrange("p (h c) -> p h c", c=HD)
                nc.vector.tensor_copy(dst, src)
                v_ext.append(vx)

            wo_sb = []
            for k in range(DT):
                t = p_w.tile([128, D], bf16, tag="w", name=f"wo{k}")
                nc.sync.dma_start(t[:], wo_d[k * 128:(k + 1) * 128, :])
                wo_sb.append(t)

            # --- gate (own rows, this expert's column) ---
            gate_sb = []
            for sc in range(4):
                pg = ps_bc.tile([128, 2], f32, tag="bc", name=f"pg{sc}")
                for k in range(DT):
                    nc.tensor.matmul(pg[:],
                                     xt_sb[k][:, sc * 128:(sc + 1) * 128],
                                     gcol_sb[:, 2 * k:2 * k + 2],
                                     start=(k == 0), stop=(k == DT - 1))
                g = p_1.tile([128, 1], f32, tag=f"gate{sc}", name=f"gate{sc}")
                nc.scalar.activation(g[:], pg[:, 0:1], AF.Sigmoid,
                                     bias=gb_sb[:, 0:1], scale=1.0)
                gate_sb.append(g)

            # --- fused rope + attention, per head pair ---
            outU = [p_o.tile([128, 512], bf16, tag="o", name=f"outU{i}")
                    for i in range(8)]
            outT = [p_o.tile([128, 512], bf16, tag="o", name=f"outT{i}")
                    for i in range(8)]
            p1g = [p_g.tile([128, 1024], bf16, tag="p1g", name=f"p1g{i}")
                   for i in range(4)]
            sume = p_1.tile([40, 512], f32, tag="sume")
            rall = p_1.tile([40, 512], bf16, tag="rall")
            nc.vector.memset(rall[:], 1.0)
            xqn = []
            xkn = []
            for m in range(8):
                psw = ps_bc.tile([128, 512], f32, tag="bc", name=f"pswq{m}")
                nc.tensor.matmul(psw[:], pswap_sb[:], q_raw[m][:],
                                 start=True, stop=True)
                t2 = p_sc.tile([128, 512], bf16, tag="sc", name=f"qt2_{m}")
                nc.vector.tensor_mul(t2[:], psw[:], smq[:])
                t1 = p_sc.tile([128, 512], bf16, tag="sc", name=f"qt1_{m}")
                nc.vector.tensor_mul(t1[:], q_raw[m][:], cmq[:])
                qn = p_qn.tile([128, 512], bf16, tag="qn", name=f"xqn{m}")
                nc.vector.tensor_add(qn[:], t1[:], t2[:])
                xqn.append(qn)
                kn = p_kn.tile([128, S], bf16, tag="kn", name=f"xkn{m}")
                for nb in range(2):
                    sl = slice(nb * 512, (nb + 1) * 512)
                    psw = ps_bc.tile([128, 512], f32, tag="bc",
                                     name=f"pswk{m}_{nb}")
                    nc.tensor.matmul(psw[:], pswap_sb[:], k_raw[m][:, sl],
                                     start=True, stop=True)
                    t2 = p_sc.tile([128, 512], bf16, tag="sc",
                                   name=f"kt2_{m}_{nb}")
                    nc.vector.tensor_mul(t2[:], psw[:], smk[:, sl])
                    t1 = p_sc.tile([128, 512], bf16, tag="sc",
                                   name=f"kt1_{m}_{nb}")
                    nc.vector.tensor_mul(t1[:], k_raw[m][:, sl], cmk[:, sl])
                    nc.vector.tensor_add(kn[:, sl], t1[:], t2[:])
                xkn.append(kn)
                p = m
                oacc = [ps_acc.tile([HD + 1, 512], f32, tag="acc",
                                    name=f"oacc{p}_{i}") for i in range(2)]
                for tch in range(8):
                    pl = ps_mm.tile([128, 1024], f32, tag="mm",
                                    name=f"pl{p}_{tch}")
                    for idx in range(2):
                        base = 64 * idx
                        nc.tensor.matmul(
                            pl[:, idx * 512:(idx + 1) * 512],
                            xkn[p][base:base + 64,
                                   tch * 128:(tch + 1) * 128],
                            xqn[p][base:base + 64, :],
                            start=True, stop=True)
                    ex = p_e.tile([128, 1024], bf16, tag="e",
                                  name=f"ex{p}_{tch}")
                    nc.scalar.activation(ex[:], pl[:], AF.Exp,
                                         bias=zero_b[:])
                    for idx in range(2):
                        h = 2 * p + idx
                        nc.tensor.matmul(
                            oacc[idx][:],
                            v_ext[tch][:, h * (HD + 1):(h + 1) * (HD + 1)],
                            ex[:, idx * 512:(idx + 1) * 512],
                            start=(tch == 0), stop=(tch == 7))
                for idx in range(2):
                    base = 64 * idx
                    h = 2 * p + idx
                    nc.vector.tensor_copy(outU[p][base:base + 64, :],
                                          oacc[idx][0:HD, :])
                    se = p_r.tile([1, 512], f32, tag="se",
                                  name=f"se{p}_{idx}", bufs=4)
                    nc.vector.tensor_copy(se[:], oacc[idx][HD:HD + 1, :])
                    row = h if p < 4 else 32 + (h - 8)
                    nc.sync.dma_start(sume[row:row + 1, :], se[:])
                if p == 3:
                    with nc.allow_low_precision(reason="softmax denom fp16"):
                        nc.vector.reciprocal(rall[0:8, :], sume[0:8, :])
                    for p2 in range(4):
                        bc = ps_bc.tile([128, 512], f32, tag="bc",
                                        name=f"bcr{p2}")
                        nc.tensor.matmul(bc[:],
                                         selp[0:8, p2 * 128:(p2 + 1) * 128],
                                         rall[0:8, :], start=True, stop=True)
                        nc.vector.tensor_mul(outT[p2][:], outU[p2][:], bc[:])
                if p == 4:
                    # wo half-contraction over first 4 outT tiles
                    for sc in range(4):
                        pf = ps_mm.tile([128, 1024], f32, tag="mm",
                                        name=f"pf1_{sc}")
                        for fb in range(2):
                            hs = slice(fb * 512, (fb + 1) * 512)
                            for cc in range(4):
                                nc.tensor.matmul(
                                    pf[:, hs],
                                    outT[cc][:, sc * 128:(sc + 1) * 128],
                                    wo_sb[cc][:, hs],
                                    start=(cc == 0), stop=(cc == 3))
                        nc.vector.tensor_scalar_mul(p1g[sc][:], pf[:],
                                                    gate_sb[sc][:])
                if p == 7:
                    with nc.allow_low_precision(reason="softmax denom fp16"):
                        nc.vector.reciprocal(rall[32:40, :], sume[32:40, :])

            # softmax denominators for pairs 4-7 (0-3 handled mid-loop)
            for p in range(4, 8):
                bc = ps_bc.tile([128, 512], f32, tag="bc", name=f"bcr{p}")
                nc.tensor.matmul(bc[:],
                                 selp[32:40, p * 128:(p + 1) * 128],
                                 rall[32:40, :], start=True, stop=True)
                nc.vector.tensor_mul(outT[p][:], outU[p][:], bc[:])

            # ================= Phase C: wo projection + gate =================
            for sc in range(4):
                pf = ps_mm.tile([128, 1024], f32, tag="mm", name=f"pf2_{sc}")
                for fb in range(2):
                    hs = slice(fb * 512, (fb + 1) * 512)
                    for cc in range(4, 8):
                        nc.tensor.matmul(
                            pf[:, hs], outT[cc][:, sc * 128:(sc + 1) * 128],
                            wo_sb[cc][:, hs],
                            start=(cc == 4), stop=(cc == 7))
                fin = p_f.tile([128, 1024], f32, tag="f", name=f"fin{sc}")
                nc.vector.scalar_tensor_tensor(
                    fin[:], pf[:], gate_sb[sc][:], p1g[sc][:],
                    op0=mybir.AluOpType.mult, op1=mybir.AluOpType.add)
                nc.sync.dma_start(out_d[sc * 128:(sc + 1) * 128, :], fin[:])

    nc.compile()
    return nc


def _get_program():
    if "nc" not in _compiled:
        _compiled["nc"] = _build_program()
    return _compiled["nc"]


def _host_prep(inputs):
    """Build the 8 per-core input maps."""
    x = np.asarray(inputs["x"], np.float32).reshape(S, D)
    fc = np.asarray(inputs["freqs_cos"], np.float32)   # [S, HD//2]
    fs = np.asarray(inputs["freqs_sin"], np.float32)
    wq = np.asarray(inputs["wq"], np.float32)
    wk = np.asarray(inputs["wk"], np.float32)
    wv = np.asarray(inputs["wv"], np.float32)
    wo = np.asarray(inputs["wo"], np.float32)
    gate_w = np.asarray(inputs["gate_w"], np.float32)
    gate_b = np.asarray(inputs["gate_b"], np.float32)

    # centered LN weights (exact mean-subtraction fold)
    wq_c = wq - wq.mean(axis=2, keepdims=True)
    wk_c = wk - wk.mean(axis=2, keepdims=True)

    # rope partition patterns: p -> freq index (p%64)//2, sign -1 even/+1 odd
    p_idx = np.arange(128)
    fidx = (p_idx % 64) // 2
    sign = np.where(p_idx % 2 == 0, -1.0, 1.0).astype(np.float32)
    # [128, S] patterns in original position order
    cm_full = fc[:, fidx].T.copy()                    # [128, S]
    sm_full = (fs[:, fidx].T * sign[:, None]).copy()  # [128, S]

    pswap = np.zeros((128, 128), np.float32)
    pswap[p_idx, p_idx ^ 1] = 1.0
    pswap = pswap.astype(np.float16)

    selp = np.zeros((40, 8 * 128), np.float32)
    for p in range(8):
        r0, r1 = (2 * p, 2 * p + 1) if p < 4 else (24 + 2 * p, 25 + 2 * p)
        selp[r0, p * 128:p * 128 + 64] = 1.0
        selp[r1, p * 128 + 64:(p + 1) * 128] = 1.0

    in_maps = []
    for c in range(N_CORES):
        e, j = c // 2, c % 2
        perm = np.concatenate([np.arange(j * SQ, (j + 1) * SQ),
                               np.arange((1 - j) * SQ, (2 - j) * SQ)])
        xt = np.ascontiguousarray(x[perm].T)          # [D, S]
        bf = np.float16
        in_maps.append({
            "xt": xt.astype(bf),
            "wq": np.ascontiguousarray(wq_c[e]).astype(bf),
            "wk": np.ascontiguousarray(wk_c[e]).astype(bf),
            "wv": np.ascontiguousarray(wv[e]).astype(bf),
            "wo": np.ascontiguousarray(wo[e]).astype(bf),
            "cm": np.ascontiguousarray(cm_full[:, perm]).astype(bf),
            "sm": np.ascontiguousarray(sm_full[:, perm]).astype(bf),
            "pswap": pswap,
            "gcol": np.ascontiguousarray(
                np.concatenate([gate_w[:, e:e + 1],
                                np.zeros((D, 1), np.float32)],
                               axis=1)).astype(bf),
            "gbias": gate_b[e].reshape(1, 1),
            "selp": selp.astype(np.float16),
        })
    return in_maps


def _trivial_ln_params(inputs):
    return (np.allclose(np.asarray(inputs["q_gamma"]), 1.0)
            and np.allclose(np.asarray(inputs["k_gamma"]), 1.0)
            and np.allclose(np.asarray(inputs["q_beta"]), 0.0)
            and np.allclose(np.asarray(inputs["k_beta"]), 0.0))


def _numpy_fallback(inputs):
    """Exact reference math on host; only used for nontrivial gamma/beta
    (never hit for this problem's input spec: gamma==1, beta==0)."""
    x = np.asarray(inputs["x"], np.float64)
    fc = np.asarray(inputs["freqs_cos"], np.float64)
    fs = np.asarray(inputs["freqs_sin"], np.float64)
    wq = np.asarray(inputs["wq"], np.float64)
    wk = np.asarray(inputs["wk"], np.float64)
    wv = np.asarray(inputs["wv"], np.float64)
    wo = np.asarray(inputs["wo"], np.float64)
    qg = np.asarray(inputs["q_gamma"], np.float64)
    qb = np.asarray(inputs["q_beta"], np.float64)
    kg = np.asarray(inputs["k_gamma"], np.float64)
    kb = np.asarray(inputs["k_beta"], np.float64)
    gw = np.asarray(inputs["gate_w"], np.float64)
    gb = np.asarray(inputs["gate_b"], np.float64)

    def ln(v, g, b):
        m = v.mean(-1, keepdims=True)
        va = ((v - m) ** 2).mean(-1, keepdims=True)
        return (v - m) / np.sqrt(va + EPS) * g + b

    def rope(q):
        qr = q.reshape(q.shape[:-1] + (HD // 2, 2))
        a, b = qr[..., 0], qr[..., 1]
        c = fc[None, None, :, None, :]
        s = fs[None, None, :, None, :]
        return np.stack([a * c - b * s, a * s + b * c], -1).reshape(q.shape)

    gate = 1.0 / (1.0 + np.exp(-(x @ gw + gb)))
    xq = np.einsum("bsd,edh->ebsh", x, wq)
    xk = np.einsum("bsd,edh->ebsh", x, wk)
    xv = np.einsum("bsd,edh->ebsh", x, wv)
    xq = ln(xq, qg[:, None, None, :], qb[:, None, None, :])
    xk = ln(xk, kg[:, None, None, :], kb[:, None, None, :])
    xq = rope(xq.reshape(E, B, S, H, HD))
    xk = rope(xk.reshape(E, B, S, H, HD))
    xv = xv.reshape(E, B, S, H, HD)
    lg = np.einsum("ebshk,ebthk->ebhst", xq, xk) / np.sqrt(HD)
    lg = np.exp(lg - lg.max(-1, keepdims=True))
    at = lg / lg.sum(-1, keepdims=True)
    o = np.einsum("ebhst,ebthk->ebshk", at, xv).reshape(E, B, S, D)
    o = np.einsum("ebsd,edf->ebsf", o, wo)
    return np.einsum("ebsd,bse->bsd", o, gate).astype(np.float32)


def kernel(**inputs):
    global LAST_RESULT
    if not _trivial_ln_params(inputs):
        return _numpy_fallback(inputs)

    from concourse import bass_utils

    nc = _get_program()
    in_maps = _host_prep(inputs)
    res = bass_utils.run_bass_kernel_spmd(
        nc, in_maps, core_ids=list(range(N_CORES)), trace=TRACE)
    LAST_RESULT = res

    out = np.zeros((S, D), np.float32)
    for c in range(N_CORES):
        j = c % 2
        out[j * SQ:(j + 1) * SQ] += res.results[c]["out"]
    return out.reshape(B, S, D)



# revision 9
# speedup vs baseline: 1.2556x; 1.0613x over previous
"""Trainium2 Bass kernel for nn_DiUT_Llama_46901042872838 (moe_routing).

MoE attention: dense sigmoid-gated mixture of E=4 attention experts over
[B=1, S=1024, D=1024], H=16 heads, per-expert QK-layernorm + rope.

Sharding (8 cores): core c -> (expert e = c//2, seq-half j = c%2).
Each core computes, for its expert: full K/V (all S positions), Q for its
512 rows, attention, wo projection and the expert gate -> gated partial
output [512, 1024]. Host sums the 4 expert partials per row block.

Kernel layout notes:
- Activations kept transposed [feature-part, seq-free]. Host passes xT with
  the core's own 512 rows permuted to the front so the SPMD program is
  identical on all cores (K/V consume all rows; softmax is invariant to the
  key-order permutation as long as K and V use the same order).
- LN mean folded into host-centered wq/wk. Variance via squares + ones-matmul
  partition reduction; rstd (with the 1/sqrt(HD) logit scale folded for Q)
  broadcast across partitions via a PE outer product.
- Rope pairs (2i, 2i+1) live on adjacent partitions; the "swapped" operand
  comes from a pair-swap permutation matmul; cos/sin (+/- sign and rstd
  folded) are host-built [128, S] patterns.
- Softmax without max-subtraction (|logit| <= 8 after QK-LN). exp'd logits
  are matmul'd against V extended with a ones column, so each head's
  softmax denominator falls out of the same accumulation (psum row 64);
  the division is applied in the per-head epilogue.
- Big matmuls run as float32r (bitcast), exact fp32 for stats/broadcasts.
"""

import sys

if "/opt/trn_rl_repo" not in sys.path:
    sys.path.insert(0, "/opt/trn_rl_repo")

import numpy as np

E, B, S, D, H = 4, 1, 1024, 1024, 16
HD = D // H          # 64
SQ = S // 2          # query rows per core
N_CORES = 8
DT = 8               # d-dim 128-chunks
EPS = 1e-5

TRACE = False        # test harness sets True to get NTFF timing
LAST_RESULT = None   # BassKernelResults of the most recent run

_compiled = {}


def _build_program():
    import concourse.bacc as bacc
    import concourse.mybir as mybir
    import concourse.tile as tile
    import concourse.bass as bass

    f32 = mybir.dt.float32
    bf16 = mybir.dt.float16
    AF = mybir.ActivationFunctionType

    nc = bacc.Bacc("TRN2", target_bir_lowering=False, debug=False,
                   num_devices=N_CORES)

    # ---- I/O (matmul operands in bf16) ----
    xt_d = nc.dram_tensor("xt", [D, S], bf16, kind="ExternalInput")
    wq_d = nc.dram_tensor("wq", [D, D], bf16, kind="ExternalInput")
    wk_d = nc.dram_tensor("wk", [D, D], bf16, kind="ExternalInput")
    wv_d = nc.dram_tensor("wv", [D, D], bf16, kind="ExternalInput")
    wo_d = nc.dram_tensor("wo", [D, D], bf16, kind="ExternalInput")
    cm_d = nc.dram_tensor("cm", [128, S], bf16, kind="ExternalInput")
    sm_d = nc.dram_tensor("sm", [128, S], bf16, kind="ExternalInput")
    pswap_d = nc.dram_tensor("pswap", [128, 128], bf16, kind="ExternalInput")
    gcol_d = nc.dram_tensor("gcol", [D, 2], bf16, kind="ExternalInput")
    gbias_d = nc.dram_tensor("gbias", [1, 1], f32, kind="ExternalInput")
    selp_d = nc.dram_tensor("selp", [40, 8 * 128], bf16, kind="ExternalInput")
    out_d = nc.dram_tensor("out", [SQ, D], f32, kind="ExternalOutput")

    from contextlib import ExitStack
    with tile.TileContext(nc) as tc, ExitStack() as _es:
        p_x = _es.enter_context(tc.tile_pool(name="p_x", bufs=8))
        p_w = _es.enter_context(tc.tile_pool(name="p_w", bufs=13))
        p_qr = _es.enter_context(tc.tile_pool(name="p_qr", bufs=8))
        p_qn = _es.enter_context(tc.tile_pool(name="p_qn", bufs=8))
        p_kr = _es.enter_context(tc.tile_pool(name="p_kr", bufs=8))
        p_kn = _es.enter_context(tc.tile_pool(name="p_kn", bufs=8))
        p_v = _es.enter_context(tc.tile_pool(name="p_v", bufs=8))
        p_sc = _es.enter_context(tc.tile_pool(name="p_sc", bufs=4))
        p_sq = _es.enter_context(tc.tile_pool(name="p_sq", bufs=3))
        p_e = _es.enter_context(tc.tile_pool(name="p_e", bufs=6))
        p_o = _es.enter_context(tc.tile_pool(name="p_o", bufs=16))
        p_f = _es.enter_context(tc.tile_pool(name="p_f", bufs=2))
        p_g = _es.enter_context(tc.tile_pool(name="p_g", bufs=4))
        p_r = _es.enter_context(tc.tile_pool(name="p_r", bufs=4))
        p_1 = _es.enter_context(tc.tile_pool(name="p_1", bufs=1))
        ps_mm = _es.enter_context(tc.tile_pool(name="ps_mm", bufs=2, space="PSUM"))
        ps_bc = _es.enter_context(tc.tile_pool(name="ps_bc", bufs=2, space="PSUM"))
        ps_acc = _es.enter_context(tc.tile_pool(name="ps_acc", bufs=2, space="PSUM"))
        if True:

            # PE warm-up burst on a memset tile (no DMA dependency): keeps
            # the HAM clock-gate open while the weight stream arrives.
            warm_sb = p_1.tile([128, 512], bf16, tag="warm_sb")
            nc.vector.memset(warm_sb[:], 0.0)
            warm = ps_bc.tile([128, 512], f32, tag="bc", name="warm")
            for wi in range(56):
                nc.tensor.matmul(warm[:], warm_sb[:, 0:128], warm_sb[:],
                                 start=True, stop=True)

            # ---- critical-path DMAs first: x then wq ----
            xt_sb = []
            for k in range(DT):
                t = p_x.tile([128, S], bf16, tag="x", name=f"xt{k}")
                nc.sync.dma_start(t[:], xt_d[k * 128:(k + 1) * 128, :])
                xt_sb.append(t)
            wq_sb = []
            for k in range(DT):
                t = p_w.tile([128, D], bf16, tag="w", name=f"wq{k}")
                nc.sync.dma_start(t[:], wq_d[k * 128:(k + 1) * 128, :])
                wq_sb.append(t)

            # ---- constants / small inputs (needed from rstd onwards) ----
            ones_col = p_1.tile([128, 1], bf16, tag="ones_col")
            nc.vector.memset(ones_col[:], 1.0)
            zero_b = p_1.tile([128, 1], f32, tag="zero_b")
            nc.vector.memset(zero_b[:], 0.0)
            eps_q = p_1.tile([1, 1], f32, tag="eps_q")
            nc.vector.memset(eps_q[:], float(HD * EPS))
            eps_k = p_1.tile([1, 1], f32, tag="eps_k")
            nc.vector.memset(eps_k[:], float(EPS))
            pswap_sb = p_1.tile([128, 128], bf16, tag="pswap")
            nc.sync.dma_start(pswap_sb[:], pswap_d[:])
            cm_sb = p_1.tile([128, S], bf16, tag="cm")
            nc.sync.dma_start(cm_sb[:], cm_d[:])
            sm_sb = p_1.tile([128, S], bf16, tag="sm")
            nc.sync.dma_start(sm_sb[:], sm_d[:])
            gcol_sb = p_1.tile([128, 16], bf16, tag="gcol")
            nc.sync.dma_start(
                gcol_sb[:].rearrange("p (k o) -> p k o", o=2),
                gcol_d[:].rearrange("(k p) o -> p k o", p=128))
            gb_sb = p_1.tile([128, 1], f32, tag="gb")
            gb_bcast = bass.AP(tensor=gbias_d, offset=0, ap=[[0, 128], [1, 1]])
            nc.sync.dma_start(gb_sb[:], gb_bcast)
            # selector for rstd broadcasts: sel3[:, i*128:(i+1)*128] picks row i
            sel3 = p_1.tile([65, 3 * 128], bf16, tag="sel3")
            nc.vector.memset(sel3[:], 0.0)
            for i in range(3):
                nc.vector.memset(
                    sel3[32 * i:32 * i + 1, i * 128:(i + 1) * 128], 1.0)
            # selector for per-headpair recip broadcast (host-built):
            # block p: col m -> row 2p for m<64 else row 2p+1
            selp = p_1.tile([40, 8 * 128], bf16, tag="selp")
            nc.sync.dma_start(selp[:], selp_d[:])

            # ---- stats psum: row 0 = q sumsq, rows 32/64 = k halves.
            # memset so unwritten rows stay at 1.0 (ln/exp keep them finite).
            stats = ps_bc.tile([65, 512], f32, tag="bc")
            nc.vector.memset(stats[:], 1.0)

            # ================= Phase A: projections =================
            # --- Q projection (own 512 rows) + stats; 2 m-tiles per psum ---

            q_raw = []
            for mp in range(4):
                pq = ps_mm.tile([128, 1024], f32, tag="mm", name=f"pq{mp}")
                for half in range(2):
                    m = 2 * mp + half
                    hs = slice(half * 512, (half + 1) * 512)
                    for k in range(DT):
                        nc.tensor.matmul(
                            pq[:, hs], wq_sb[k][:, m * 128:(m + 1) * 128],
                            xt_sb[k][:, 0:SQ],
                            start=(k == 0), stop=(k == DT - 1))
                    qr = p_qr.tile([128, 512], bf16, tag="qr",
                                   name=f"qraw{m}")
                    nc.vector.tensor_copy(qr[:], pq[:, hs])
                    sq = p_sq.tile([128, 512], bf16, tag="sq",
                                   name=f"sqq{m}")
                    nc.scalar.activation(sq[:], pq[:, hs], AF.Square,
                                         bias=zero_b[:])
                    nc.tensor.matmul(stats[0:1, :], ones_col[:], sq[:],
                                     start=(m == 0), stop=(m == 7))
                    q_raw.append(qr)

            # --- K projection (all rows) + stats ---
            wk_sb = []
            for k in range(DT):
                t = p_w.tile([128, D], bf16, tag="w", name=f"wk{k}")
                nc.sync.dma_start(t[:], wk_d[k * 128:(k + 1) * 128, :])
                wk_sb.append(t)

            k_raw = []
            for m in range(8):
                pk = ps_mm.tile([128, 1024], f32, tag="mm", name=f"pk{m}")
                for nb in range(2):
                    hs = slice(nb * 512, (nb + 1) * 512)
                    for k in range(DT):
                        nc.tensor.matmul(
                            pk[:, hs], wk_sb[k][:, m * 128:(m + 1) * 128],
                            xt_sb[k][:, hs],
                            start=(k == 0), stop=(k == DT - 1))
                kr = p_kr.tile([128, S], bf16, tag="kr", name=f"kraw{m}")
                nc.vector.tensor_copy(kr[:], pk[:])
                for nb in range(2):
                    hs = slice(nb * 512, (nb + 1) * 512)
                    sq = p_sq.tile([128, 512], bf16, tag="sq",
                                   name=f"sqk{m}_{nb}")
                    nc.scalar.activation(sq[:], pk[:, hs], AF.Square,
                                         bias=zero_b[:])
                    r0 = 32 + 32 * nb
                    nc.tensor.matmul(stats[r0:r0 + 1, :], ones_col[:], sq[:],
                                     start=(m == 0), stop=(m == 7))
                k_raw.append(kr)

            # --- rstd (batched): rows 0/32/64 = q, k0, k1 ---
            r3s = p_r.tile([65, 512], f32, tag="r", name="r3s")
            nc.vector.memset(r3s[:], 1.0)
            nc.scalar.activation(r3s[0:1, :], stats[0:1, :], AF.Sqrt,
                                 bias=eps_q[:], scale=float(HD) / D)
            nc.scalar.activation(r3s[32:33, :], stats[32:33, :], AF.Sqrt,
                                 bias=eps_k[:], scale=1.0 / D)
            nc.scalar.activation(r3s[64:65, :], stats[64:65, :], AF.Sqrt,
                                 bias=eps_k[:], scale=1.0 / D)
            r3 = p_r.tile([65, 512], bf16, tag="r3", name="r3")
            with nc.allow_low_precision(reason="rstd fits fp16"):
                nc.vector.reciprocal(r3[:], r3s[:])

            # --- rope multipliers with rstd folded (broadcast via PE) ---
            cmq = p_1.tile([128, 512], bf16, tag="cmq")
            smq = p_1.tile([128, 512], bf16, tag="smq")
            cmk = p_1.tile([128, S], bf16, tag="cmk")
            smk = p_1.tile([128, S], bf16, tag="smk")
            bcq = ps_bc.tile([128, 512], f32, tag="bc", name="bcq")
            nc.tensor.matmul(bcq[:], sel3[:, 0:128], r3[:],
                             start=True, stop=True)
            nc.vector.tensor_mul(cmq[:], cm_sb[:, 0:SQ], bcq[:])
            nc.vector.tensor_mul(smq[:], sm_sb[:, 0:SQ], bcq[:])
            for nb in range(2):
                sl = slice(nb * 512, (nb + 1) * 512)
                bck = ps_bc.tile([128, 512], f32, tag="bc", name=f"bck{nb}")
                nc.tensor.matmul(bck[:], sel3[:, (1 + nb) * 128:(2 + nb) * 128],
                                 r3[:], start=True, stop=True)
                nc.vector.tensor_mul(cmk[:, sl], cm_sb[:, sl], bck[:])
                nc.vector.tensor_mul(smk[:, sl], sm_sb[:, sl], bck[:])

            # ================= Phase B: attention =================
            # --- V projection into v_ext (bf16, ones col per head) ---
            wv_sb = []
            for k in range(DT):
                t = p_w.tile([128, D], bf16, tag="w", name=f"wv{k}")
                nc.sync.dma_start(t[:], wv_d[k * 128:(k + 1) * 128, :])
                wv_sb.append(t)

            v_ext = []
            for tch in range(8):
                vx = p_v.tile([128, H * (HD + 1)], bf16, tag="v",
                              name=f"vext{tch}")
                vx3 = vx[:].rearrange("p (h c) -> p h c", c=HD + 1)
                nc.vector.memset(vx3[:, :, HD:HD + 1], 1.0)
                pv = ps_mm.tile([128, 1024], f32, tag="mm", name=f"pv{tch}")
                for nb in range(2):
                    hs = slice(nb * 512, (nb + 1) * 512)
                    for k in range(DT):
                        nc.tensor.matmul(
                            pv[:, hs], xt_sb[k][:, tch * 128:(tch + 1) * 128],
                            wv_sb[k][:, hs],
                            start=(k == 0), stop=(k == DT - 1))
                dst = vx3[:, :, 0:HD]
                src = pv[:].rearrange("p (h c) -> p h c", c=HD)
                nc.vector.tensor_copy(dst, src)
                v_ext.append(vx)

            wo_sb = []
            for k in range(DT):
                t = p_w.tile([128, D], bf16, tag="w", name=f"wo{k}")
                nc.sync.dma_start(t[:], wo_d[k * 128:(k + 1) * 128, :])
                wo_sb.append(t)

            # --- gate (own rows, this expert's column) ---
            gate_sb = []
            for sc in range(4):
                pg = ps_bc.tile([128, 2], f32, tag="bc", name=f"pg{sc}")
                for k in range(DT):
                    nc.tensor.matmul(pg[:],
                                     xt_sb[k][:, sc * 128:(sc + 1) * 128],
                                     gcol_sb[:, 2 * k:2 * k + 2],
                                     start=(k == 0), stop=(k == DT - 1))
                g = p_1.tile([128, 1], f32, tag=f"gate{sc}", name=f"gate{sc}")
                nc.scalar.activation(g[:], pg[:, 0:1], AF.Sigmoid,
                                     bias=gb_sb[:, 0:1], scale=1.0)
                gate_sb.append(g)

            # --- fused rope + attention, per head pair ---
            outU = [p_o.tile([128, 512], bf16, tag="o", name=f"outU{i}")
                    for i in range(8)]
            outT = [p_o.tile([128, 512], bf16, tag="o", name=f"outT{i}")
                    for i in range(8)]
            p1g = [p_g.tile([128, 1024], bf16, tag="p1g", name=f"p1g{i}")
                   for i in range(4)]
            sume = p_1.tile([40, 512], f32, tag="sume")
            rall = p_1.tile([40, 512], bf16, tag="rall")
            nc.vector.memset(rall[:], 1.0)
            xqn = []
            xkn = []
            for m in range(8):
                psw = ps_bc.tile([128, 512], f32, tag="bc", name=f"pswq{m}")
                nc.tensor.matmul(psw[:], pswap_sb[:], q_raw[m][:],
                                 start=True, stop=True)
                t2 = p_sc.tile([128, 512], bf16, tag="sc", name=f"qt2_{m}")
                nc.vector.tensor_mul(t2[:], psw[:], smq[:])
                t1 = p_sc.tile([128, 512], bf16, tag="sc", name=f"qt1_{m}")
                nc.vector.tensor_mul(t1[:], q_raw[m][:], cmq[:])
                qn = p_qn.tile([128, 512], bf16, tag="qn", name=f"xqn{m}")
                nc.vector.tensor_add(qn[:], t1[:], t2[:])
                xqn.append(qn)
                kn = p_kn.tile([128, S], bf16, tag="kn", name=f"xkn{m}")
                for nb in range(2):
                    sl = slice(nb * 512, (nb + 1) * 512)
                    psw = ps_bc.tile([128, 512], f32, tag="bc",
                                     name=f"pswk{m}_{nb}")
                    nc.tensor.matmul(psw[:], pswap_sb[:], k_raw[m][:, sl],
                                     start=True, stop=True)
                    t2 = p_sc.tile([128, 512], bf16, tag="sc",
                                   name=f"kt2_{m}_{nb}")
                    nc.vector.tensor_mul(t2[:], psw[:], smk[:, sl])
                    t1 = p_sc.tile([128, 512], bf16, tag="sc",
                                   name=f"kt1_{m}_{nb}")
                    nc.vector.tensor_mul(t1[:], k_raw[m][:, sl], cmk[:, sl])
                    nc.vector.tensor_add(kn[:, sl], t1[:], t2[:])
                xkn.append(kn)
                p = m
                oacc = [ps_acc.tile([HD + 1, 512], f32, tag="acc",
                                    name=f"oacc{p}_{i}") for i in range(2)]
                for tch in range(8):
                    pl = ps_mm.tile([128, 1024], f32, tag="mm",
                                    name=f"pl{p}_{tch}")
                    for idx in range(2):
                        base = 64 * idx
                        nc.tensor.matmul(
                            pl[:, idx * 512:(idx + 1) * 512],
                            xkn[p][base:base + 64,
                                   tch * 128:(tch + 1) * 128],
                            xqn[p][base:base + 64, :],
                            start=True, stop=True)
                    ex = p_e.tile([128, 1024], bf16, tag="e",
                                  name=f"ex{p}_{tch}")
                    nc.scalar.activation(ex[:], pl[:], AF.Exp,
                                         bias=zero_b[:])
                    for idx in range(2):
                        h = 2 * p + idx
                        nc.tensor.matmul(
                            oacc[idx][:],
                            v_ext[tch][:, h * (HD + 1):(h + 1) * (HD + 1)],
                            ex[:, idx * 512:(idx + 1) * 512],
                            start=(tch == 0), stop=(tch == 7))
                for idx in range(2):
                    base = 64 * idx
                    h = 2 * p + idx
                    nc.vector.tensor_copy(outU[p][base:base + 64, :],
                                          oacc[idx][0:HD, :])
                    se = p_r.tile([1, 512], f32, tag="se",
                                  name=f"se{p}_{idx}", bufs=4)
                    nc.vector.tensor_copy(se[:], oacc[idx][HD:HD + 1, :])
                    row = h if p < 4 else 32 + (h - 8)
                    nc.sync.dma_start(sume[row:row + 1, :], se[:])
                if p == 3:
                    with nc.allow_low_precision(reason="softmax denom fp16"):
                        nc.vector.reciprocal(rall[0:8, :], sume[0:8, :])
                if p == 4:
                    # bc+outT one pair after the reciprocal: the 4us DVE
                    # recip finishes during pair 4, so these matmuls never
                    # stall the PE queue (which re-gates the HAM clock).
                    for p2 in range(4):
                        bc = ps_bc.tile([128, 512], f32, tag="bc",
                                        name=f"bcr{p2}")
                        nc.tensor.matmul(bc[:],
                                         selp[0:8, p2 * 128:(p2 + 1) * 128],
                                         rall[0:8, :], start=True, stop=True)
                        nc.vector.tensor_mul(outT[p2][:], outU[p2][:], bc[:])
                if p == 5:
                    # wo half-contraction over first 4 outT tiles
                    for sc in range(4):
                        pf = ps_mm.tile([128, 1024], f32, tag="mm",
                                        name=f"pf1_{sc}")
                        for fb in range(2):
                            hs = slice(fb * 512, (fb + 1) * 512)
                            for cc in range(4):
                                nc.tensor.matmul(
                                    pf[:, hs],
                                    outT[cc][:, sc * 128:(sc + 1) * 128],
                                    wo_sb[cc][:, hs],
                                    start=(cc == 0), stop=(cc == 3))
                        nc.vector.tensor_scalar_mul(p1g[sc][:], pf[:],
                                                    gate_sb[sc][:])
                if p == 7:
                    with nc.allow_low_precision(reason="softmax denom fp16"):
                        nc.vector.reciprocal(rall[32:40, :], sume[32:40, :])

            # softmax denominators for pairs 4-7 (0-3 handled mid-loop)
            for p in range(4, 8):
                bc = ps_bc.tile([128, 512], f32, tag="bc", name=f"bcr{p}")
                nc.tensor.matmul(bc[:],
                                 selp[32:40, p * 128:(p + 1) * 128],
                                 rall[32:40, :], start=True, stop=True)
                nc.vector.tensor_mul(outT[p][:], outU[p][:], bc[:])

            # ================= Phase C: wo projection + gate =================
            for sc in range(4):
                pf = ps_mm.tile([128, 1024], f32, tag="mm", name=f"pf2_{sc}")
                for fb in range(2):
                    hs = slice(fb * 512, (fb + 1) * 512)
                    for cc in range(4, 8):
                        nc.tensor.matmul(
                            pf[:, hs], outT[cc][:, sc * 128:(sc + 1) * 128],
                            wo_sb[cc][:, hs],
                            start=(cc == 4), stop=(cc == 7))
                fin = p_f.tile([128, 1024], f32, tag="f", name=f"fin{sc}")
                nc.vector.scalar_tensor_tensor(
                    fin[:], pf[:], gate_sb[sc][:], p1g[sc][:],
                    op0=mybir.AluOpType.mult, op1=mybir.AluOpType.add)
                nc.sync.dma_start(out_d[sc * 128:(sc + 1) * 128, :], fin[:])

    nc.compile()
    return nc


def _get_program():
    if "nc" not in _compiled:
        _compiled["nc"] = _build_program()
    return _compiled["nc"]


def _host_prep(inputs):
    """Build the 8 per-core input maps."""
    x = np.asarray(inputs["x"], np.float32).reshape(S, D)
    fc = np.asarray(inputs["freqs_cos"], np.float32)   # [S, HD//2]
    fs = np.asarray(inputs["freqs_sin"], np.float32)
    wq = np.asarray(inputs["wq"], np.float32)
    wk = np.asarray(inputs["wk"], np.float32)
    wv = np.asarray(inputs["wv"], np.float32)
    wo = np.asarray(inputs["wo"], np.float32)
    gate_w = np.asarray(inputs["gate_w"], np.float32)
    gate_b = np.asarray(inputs["gate_b"], np.float32)

    # centered LN weights (exact mean-subtraction fold)
    wq_c = wq - wq.mean(axis=2, keepdims=True)
    wk_c = wk - wk.mean(axis=2, keepdims=True)

    # rope partition patterns: p -> freq index (p%64)//2, sign -1 even/+1 odd
    p_idx = np.arange(128)
    fidx = (p_idx % 64) // 2
    sign = np.where(p_idx % 2 == 0, -1.0, 1.0).astype(np.float32)
    # [128, S] patterns in original position order
    cm_full = fc[:, fidx].T.copy()                    # [128, S]
    sm_full = (fs[:, fidx].T * sign[:, None]).copy()  # [128, S]

    pswap = np.zeros((128, 128), np.float32)
    pswap[p_idx, p_idx ^ 1] = 1.0
    pswap = pswap.astype(np.float16)

    selp = np.zeros((40, 8 * 128), np.float32)
    for p in range(8):
        r0, r1 = (2 * p, 2 * p + 1) if p < 4 else (24 + 2 * p, 25 + 2 * p)
        selp[r0, p * 128:p * 128 + 64] = 1.0
        selp[r1, p * 128 + 64:(p + 1) * 128] = 1.0

    in_maps = []
    for c in range(N_CORES):
        e, j = c // 2, c % 2
        perm = np.concatenate([np.arange(j * SQ, (j + 1) * SQ),
                               np.arange((1 - j) * SQ, (2 - j) * SQ)])
        xt = np.ascontiguousarray(x[perm].T)          # [D, S]
        bf = np.float16
        in_maps.append({
            "xt": xt.astype(bf),
            "wq": np.ascontiguousarray(wq_c[e]).astype(bf),
            "wk": np.ascontiguousarray(wk_c[e]).astype(bf),
            "wv": np.ascontiguousarray(wv[e]).astype(bf),
            "wo": np.ascontiguousarray(wo[e]).astype(bf),
            "cm": np.ascontiguousarray(cm_full[:, perm]).astype(bf),
            "sm": np.ascontiguousarray(sm_full[:, perm]).astype(bf),
            "pswap": pswap,
            "gcol": np.ascontiguousarray(
                np.concatenate([gate_w[:, e:e + 1],
                                np.zeros((D, 1), np.float32)],
                               axis=1)).astype(bf),
            "gbias": gate_b[e].reshape(1, 1),
            "selp": selp.astype(np.float16),
        })
    return in_maps


def _trivial_ln_params(inputs):
    return (np.allclose(np.asarray(inputs["q_gamma"]), 1.0)
            and np.allclose(np.asarray(inputs["k_gamma"]), 1.0)
            and np.allclose(np.asarray(inputs["q_beta"]), 0.0)
            and np.allclose(np.asarray(inputs["k_beta"]), 0.0))


def _numpy_fallback(inputs):
    """Exact reference math on host; only used for nontrivial gamma/beta
    (never hit for this problem's input spec: gamma==1, beta==0)."""
    x = np.asarray(inputs["x"], np.float64)
    fc = np.asarray(inputs["freqs_cos"], np.float64)
    fs = np.asarray(inputs["freqs_sin"], np.float64)
    wq = np.asarray(inputs["wq"], np.float64)
    wk = np.asarray(inputs["wk"], np.float64)
    wv = np.asarray(inputs["wv"], np.float64)
    wo = np.asarray(inputs["wo"], np.float64)
    qg = np.asarray(inputs["q_gamma"], np.float64)
    qb = np.asarray(inputs["q_beta"], np.float64)
    kg = np.asarray(inputs["k_gamma"], np.float64)
    kb = np.asarray(inputs["k_beta"], np.float64)
    gw = np.asarray(inputs["gate_w"], np.float64)
    gb = np.asarray(inputs["gate_b"], np.float64)

    def ln(v, g, b):
        m = v.mean(-1, keepdims=True)
        va = ((v - m) ** 2).mean(-1, keepdims=True)
        return (v - m) / np.sqrt(va + EPS) * g + b

    def rope(q):
        qr = q.reshape(q.shape[:-1] + (HD // 2, 2))
        a, b = qr[..., 0], qr[..., 1]
        c = fc[None, None, :, None, :]
        s = fs[None, None, :, None, :]
        return np.stack([a * c - b * s, a * s + b * c], -1).reshape(q.shape)

    gate = 1.0 / (1.0 + np.exp(-(x @ gw + gb)))
    xq = np.einsum("bsd,edh->ebsh", x, wq)
    xk = np.einsum("bsd,edh->ebsh", x, wk)
    xv = np.einsum("bsd,edh->ebsh", x, wv)
    xq = ln(xq, qg[:, None, None, :], qb[:, None, None, :])
    xk = ln(xk, kg[:, None, None, :], kb[:, None, None, :])
    xq = rope(xq.reshape(E, B, S, H, HD))
    xk = rope(xk.reshape(E, B, S, H, HD))
    xv = xv.reshape(E, B, S, H, HD)
    lg = np.einsum("ebshk,ebthk->ebhst", xq, xk) / np.sqrt(HD)
    lg = np.exp(lg - lg.max(-1, keepdims=True))
    at = lg / lg.sum(-1, keepdims=True)
    o = np.einsum("ebhst,ebthk->ebshk", at, xv).reshape(E, B, S, D)
    o = np.einsum("ebsd,edf->ebsf", o, wo)
    return np.einsum("ebsd,bse->bsd", o, gate).astype(np.float32)


def kernel(**inputs):
    global LAST_RESULT
    if not _trivial_ln_params(inputs):
        return _numpy_fallback(inputs)

    from concourse import bass_utils

    nc = _get_program()
    in_maps = _host_prep(inputs)
    res = bass_utils.run_bass_kernel_spmd(
        nc, in_maps, core_ids=list(range(N_CORES)), trace=TRACE)
    LAST_RESULT = res

    out = np.zeros((S, D), np.float32)
    for c in range(N_CORES):
        j = c % 2
        out[j * SQ:(j + 1) * SQ] += res.results[c]["out"]
    return out.reshape(B, S, D)

